# revision 13
# baseline (speedup 1.0000x reference)
"""Multi-head attention (B=8, T=2048, D=512, H=8) on 8 TRN2 NeuronCores.

Sharding: data-parallel over batch - one batch element per core, no
collectives. Host-side prep: transpose x inputs to [D, T], cast matmul
operands to bf16, pass (1 - mask)^T chunk-major, transpose per-core y^T
back to [T, D] (y travels bf16; host converts to f32).

The kernel is organized around keeping the Scalar (activation) engine's
exp stream gapless: 256 exp slices of [128, 1024] at ~1us each are the
hard floor. Everything else (projections, attn-V, output proj,
epilogues) is scheduled into the PE/DVE/DMA slack around that stream:

  P1: PE p-state warmup, then V = x Wv^T + bv -> vaug tiles (ones
      column for the softmax denominator), with K j=0,1 and Q j=0
      projections interleaved as their DMAs land.
  P2: one global pipeline over (qb, h, c):
        S^T[t2,q] = K_h^T.T @ Q_h^T       (64-contraction matmul at
                                           partition base (h%2)*64; PSUM
                                           "s" 2-slot rotation)
        P_raw     = exp(S^T / 8)          (ScalarE, the critical stream,
                                           into [128,2048] chunk pairs)
        P         = P_raw * (1-mask)^T    (DVE, one paired [128,2048]
                                           multiply per 2 chunks)
        O_aug^T  += Vaug_h.T @ P          (PSUM "o" banks, global 3-chunk
                                           lag via a FIFO)
      Remaining K/Q projections and P3(qb=0) y-tiles are injected into
      the "o"-parity PSUM banks mid-head so they never stall the "s"
      rotation. The head epilogue (approx-reciprocal of the denominator
      row + normalize into o2) is emitted as 3 stages interleaved into
      the NEXT head's chunks so DVE never idles on its DMA-latency chain.
  P3: y^T = Wo^T.T @ O^T (+bo) per fo-tile; qb=0's tiles injected during
      qb=1's middle heads; qb=1's in the tail with the j<3 partial
      matmuls hoisted before the final epilogue and keep-warm matmuls
      covering the epilogue's DMA latency.

No max-subtraction in softmax: scores are O(6) so exp is safe in f32.
"""

import numpy as np
import ml_dtypes

B, T, FDIM, H = 8, 2048, 512, 8
DK = FDIM // H          # 64
NFT = FDIM // 128       # 4 fo-tiles
NCH = T // 128          # 16 t2-chunks
QB = 2                  # q blocks
QBS = T // QB           # 1024
N_CORES = 8
LAG = 3                 # attnV trails the exp stream by LAG chunks (global)

BF16 = ml_dtypes.bfloat16

_cache = {}


def _build_nc():
    import concourse.bass as bass
    import concourse.mybir as mybir
    from concourse import bacc, tile

    f32 = mybir.dt.float32
    bf16 = mybir.dt.bfloat16
    Exp = mybir.ActivationFunctionType.Exp
    Alu = mybir.AluOpType

    nc = bacc.Bacc("TRN2", target_bir_lowering=False, debug=False,
                   num_devices=N_CORES)

    # DRAM I/O (per-core shard shapes)
    xqT = nc.dram_tensor("xqT", [FDIM, T], bf16, kind="ExternalInput")
    xkT = nc.dram_tensor("xkT", [FDIM, T], bf16, kind="ExternalInput")
    xvT = nc.dram_tensor("xvT", [FDIM, T], bf16, kind="ExternalInput")
    wqT = nc.dram_tensor("wqT", [FDIM, FDIM], bf16, kind="ExternalInput")
    wkT = nc.dram_tensor("wkT", [FDIM, FDIM], bf16, kind="ExternalInput")
    wvT = nc.dram_tensor("wvT", [FDIM, FDIM], bf16, kind="ExternalInput")
    woT = nc.dram_tensor("woT", [FDIM, FDIM], bf16, kind="ExternalInput")
    bq = nc.dram_tensor("bq", [FDIM], f32, kind="ExternalInput")
    bk = nc.dram_tensor("bk", [FDIM], f32, kind="ExternalInput")
    bv = nc.dram_tensor("bv", [FDIM], f32, kind="ExternalInput")
    bo = nc.dram_tensor("bo", [FDIM], f32, kind="ExternalInput")
    mbar = nc.dram_tensor("mbar", [NCH, 128, T], bf16, kind="ExternalInput")
    yT = nc.dram_tensor("yT", [FDIM, T], bf16, kind="ExternalOutput")
    # DRAM bounce rows for partition-broadcasting softmax reciprocals
    rscratch = nc.dram_tensor("rscratch", [QB * H, QBS], f32)

    with tile.TileContext(nc) as tc:
        with (
            tc.tile_pool(name="consts", bufs=1) as consts,
            tc.tile_pool(name="qt", bufs=1) as qt_pool,
            tc.tile_pool(name="kpad", bufs=1) as kpad_pool,
            tc.tile_pool(name="vaug", bufs=1) as vaug_pool,
            tc.tile_pool(name="osb", bufs=1) as osb_pool,
            tc.tile_pool(name="ysb", bufs=2) as ysb_pool,
            tc.tile_pool(name="xt", bufs=4) as xt_pool,
            tc.tile_pool(name="mask", bufs=8) as mask_pool,
            tc.tile_pool(name="praw", bufs=2) as praw_pool,
            tc.tile_pool(name="pm", bufs=3) as pm_pool,
            tc.tile_pool(name="rb", bufs=1) as rb_pool,
            tc.tile_pool(name="psum", bufs=2, space="PSUM") as psum_pool,
        ):
            # ---- DMA staging, critical path first ----
            wv_sb = [consts.tile([128, FDIM], bf16, tag=f"wv{fc}", name=f"wv{fc}") for fc in range(4)]
            wk_sb = [consts.tile([128, FDIM], bf16, tag=f"wk{fc}", name=f"wk{fc}") for fc in range(4)]
            wq_sb = [consts.tile([128, FDIM], bf16, tag=f"wq{fc}", name=f"wq{fc}") for fc in range(4)]
            wo_sb = [consts.tile([128, FDIM], bf16, tag=f"wo{j}", name=f"wo{j}") for j in range(NFT)]
            bq_sb = consts.tile([128, NFT], f32, tag="bq", name="bq")
            bk_sb = consts.tile([128, NFT], f32, tag="bk", name="bk")
            bo_sb = consts.tile([128, NFT], f32, tag="bo", name="bo")
            bv_bcast = consts.tile([128, FDIM], f32, tag="bv_bcast", name="bv_bcast")

            for fc in range(4):
                nc.sync.dma_start(out=wv_sb[fc][:], in_=wvT[fc * 128:(fc + 1) * 128, :])
            nc.sync.dma_start(
                out=bv_bcast[:],
                in_=bv.ap().rearrange("(a f) -> a f", a=1).to_broadcast([128, FDIM]))
            xts_v = []
            for fc in range(4):
                xt = xt_pool.tile([128, T], bf16, tag="xv", bufs=4, name="xv")
                nc.sync.dma_start(out=xt[:], in_=xvT[fc * 128:(fc + 1) * 128, :])
                xts_v.append(xt)
            for fc in range(4):
                nc.sync.dma_start(out=wk_sb[fc][:], in_=wkT[fc * 128:(fc + 1) * 128, :])
            xts_k = []
            for fc in range(4):
                xt = xt_pool.tile([128, T], bf16, tag="xk", bufs=4, name="xk")
                nc.sync.dma_start(out=xt[:], in_=xkT[fc * 128:(fc + 1) * 128, :])
                xts_k.append(xt)
            for b_dram, b_t in ((bk, bk_sb), (bq, bq_sb)):
                nc.sync.dma_start(out=b_t[:], in_=b_dram.ap().rearrange("(j p) -> p j", p=128))
            for fc in range(4):
                nc.sync.dma_start(out=wq_sb[fc][:], in_=wqT[fc * 128:(fc + 1) * 128, :])
            xts_q = []
            for fc in range(4):
                xt = xt_pool.tile([128, T], bf16, tag="xq", bufs=4, name="xq")
                nc.sync.dma_start(out=xt[:], in_=xqT[fc * 128:(fc + 1) * 128, :])
                xts_q.append(xt)
            nc.sync.dma_start(out=bo_sb[:], in_=bo.ap().rearrange("(j p) -> p j", p=128))
            for j in range(NFT):
                nc.sync.dma_start(out=wo_sb[j][:], in_=woT[j * 128:(j + 1) * 128, :])

            # ---- persistent activation tiles ----
            # kpad[j]: rows 0:64 = K^T of head 2j, rows 64:128 = head 2j+1.
            # Scores use 64-partition matmuls at base (h%2)*64 - no zero
            # padding needed.
            qT_sb = [qt_pool.tile([128, T], bf16, tag=f"qT{j}", name=f"qT{j}") for j in range(NFT)]
            kpad = [kpad_pool.tile([128, T], bf16, tag=f"kp{j}", name=f"kp{j}") for j in range(NFT)]
            vaug = [vaug_pool.tile([128, H * (DK + 1)], bf16, tag=f"va{tt}", name=f"va{tt}")
                    for tt in range(NCH)]
            for tt in range(NCH):
                va = vaug[tt][:].rearrange("p (h d) -> p h d", d=DK + 1)
                nc.vector.memset(va[:, :, DK:DK + 1], 1.0)

            o2_sb = {}
            for qb in range(QB):
                for j in range(NFT):
                    o2_sb[(qb, j)] = osb_pool.tile([128, QBS], bf16, tag=f"o2_{qb}_{j}",
                                                   name=f"o2_{qb}_{j}")

            # ---------------- unit emitters ----------------
            # PSUM: tag "s" = 2x[128,1024] (4 banks, the exp-feed rotation)
            #       tag o0/o1 = 1 each (2 banks each: o-accum / injected units)

            def warm(n, tag="s"):
                # p-state keep-warm: junk matmuls, results never read
                for _ in range(n):
                    wps = psum_pool.tile([128, 512], mybir.dt.float32,
                                         tag=tag, name="warm")
                    nc.tensor.matmul(wps[:], wv_sb[0][:, 0:128], wv_sb[0][:],
                                     start=True, stop=True)

            def v_unit(tt):
                ps = psum_pool.tile([128, 512], mybir.dt.float32,
                                    tag=f"o{tt % 2}", bufs=1, name="vp")
                for fc in range(4):
                    nc.tensor.matmul(
                        ps[:],
                        xts_v[fc][:, tt * 128:(tt + 1) * 128],
                        wv_sb[fc][:],
                        start=(fc == 0), stop=(fc == 3),
                    )
                va = vaug[tt][:].rearrange("p (h d) -> p h d", d=DK + 1)
                nc.vector.scalar_tensor_tensor(
                    out=va[:, :, 0:DK],
                    in0=ps[:].rearrange("p (h d) -> p h d", d=DK),
                    scalar=1.0,
                    in1=bv_bcast[:].rearrange("p (h d) -> p h d", d=DK),
                    op0=Alu.mult, op1=Alu.add,
                )

            def qk_unit(kind, j, s, ptag):
                xts, w_sb, b_t = ((xts_q, wq_sb, bq_sb) if kind == "q"
                                  else (xts_k, wk_sb, bk_sb))
                ps = psum_pool.tile([128, 512], mybir.dt.float32,
                                    tag=ptag, bufs=(2 if ptag == "s" else 1),
                                    name="qkp")
                for fc in range(4):
                    nc.tensor.matmul(
                        ps[:],
                        w_sb[fc][:, j * 128:(j + 1) * 128],
                        xts[fc][:, s * 512:(s + 1) * 512],
                        start=(fc == 0), stop=(fc == 3),
                    )
                sl = slice(s * 512, (s + 1) * 512)
                if kind == "q":
                    nc.vector.tensor_scalar_add(qT_sb[j][:, sl], ps[:],
                                                b_t[:, j:j + 1])
                else:
                    nc.vector.tensor_scalar_add(
                        kpad[j][0:64, sl], ps[0:64, :], b_t[0:64, j:j + 1])
                    nc.vector.tensor_scalar_add(
                        kpad[j][64:128, sl], ps[64:128, :],
                        b_t[64:128, j:j + 1])

            def p3_unit(qb, i, ptag, js=range(NFT), y_ps=None, finish=True):
                # output-projection fo-tile i for q-block qb over head-pair
                # tiles `js`; returns the psum tile when not finished.
                qsl = slice(qb * QBS, (qb + 1) * QBS)
                if y_ps is None:
                    y_ps = psum_pool.tile([128, QBS], mybir.dt.float32,
                                          tag=ptag, bufs=1, name="y")
                js = list(js)
                for j in js:
                    for s in range(2):
                        nc.tensor.matmul(
                            y_ps[:, s * 512:(s + 1) * 512],
                            wo_sb[j][:, i * 128:(i + 1) * 128],
                            o2_sb[(qb, j)][:, s * 512:(s + 1) * 512],
                            start=(j == 0), stop=(finish and j == NFT - 1),
                        )
                if not finish:
                    return y_ps
                y_sb = ysb_pool.tile([128, QBS], bf16, tag="ysb", name="ysb")
                nc.vector.tensor_scalar_add(y_sb[:], y_ps[:], bo_sb[:, i:i + 1])
                nc.sync.dma_start(out=yT[i * 128:(i + 1) * 128, qsl], in_=y_sb[:])
                return None

            # ---------------- P1 ----------------
            warm(20)
            p1_stream = [("v", tt) for tt in range(NCH)]
            p1_stream[5:5] = [("k", 0, 0), ("k", 0, 1), ("k", 0, 2), ("k", 0, 3)]
            p1_stream[13:13] = [("k", 1, 0), ("k", 1, 1), ("q", 0, 0)]
            p1_stream.extend([("k", 1, 2), ("k", 1, 3), ("q", 0, 1)])
            for u in p1_stream:
                if u[0] == "v":
                    v_unit(u[1])
                else:
                    qk_unit(u[0], u[1], u[2], ptag="s")

            # ---------------- P2 ----------------
            # deadlines: k(j)/q(j, qb-halves) complete before head 2j of
            # that q-block (k(0), k(1), q(0) qb0-half done in P1).
            inj = {qb: {h: [] for h in range(H)} for qb in range(QB)}
            inj[0][0] = [("q", 1, 0), ("q", 1, 1)]
            inj[0][1] = [("k", 2, 0), ("k", 2, 1)]
            inj[0][2] = [("k", 2, 2), ("k", 2, 3)]
            inj[0][3] = [("q", 2, 0), ("q", 2, 1)]
            inj[0][4] = [("k", 3, 0), ("k", 3, 1), ("k", 3, 2)]
            inj[0][5] = [("k", 3, 3), ("q", 3, 0), ("q", 3, 1)]
            inj[0][6] = [("q", 0, 2)]
            inj[0][7] = [("q", 0, 3)]
            inj[1][0] = [("q", 1, 2), ("q", 1, 3)]
            inj[1][1] = [("q", 2, 2), ("q", 2, 3)]
            inj[1][2] = [("q", 3, 2), ("q", 3, 3)]
            inj[1][3] = [("p3", 0, 0)]
            inj[1][4] = [("p3", 0, 1)]
            inj[1][5] = [("p3", 0, 2)]
            inj[1][6] = [("p3", 0, 3)]

            mask_t = {}

            def load_masks(qb, pairs):
                for p in pairs:
                    mt = mask_pool.tile([128, 2 * QBS], bf16, tag="mask", name="mask")
                    for k in range(2):
                        nc.sync.dma_start(
                            out=mt[:, k * QBS:(k + 1) * QBS],
                            in_=mbar[2 * p + k, :, qb * QBS:(qb + 1) * QBS])
                    mask_t[(qb, p)] = mt

            load_masks(0, range(NCH // 2))

            def epilogue_stages(qb, h, o_ps):
                j, lo = h // 2, (h % 2) * 64
                rrow = rscratch.ap()[qb * H + h: qb * H + h + 1, :]
                rb = rb_pool.tile([128, QBS], mybir.dt.float32, tag="rb", bufs=1, name="rb")
                rbs = rb_pool.tile([8, QBS // 8], mybir.dt.float32, tag="rbs", bufs=1, name="rbs")
                rbr = rb_pool.tile([8, QBS // 8], mybir.dt.float32, tag="rbr", bufs=1, name="rbr")

                def s1():
                    nc.vector.tensor_copy(rb[64:65, :], o_ps[DK:DK + 1, :])
                    nc.sync.dma_start(out=rbs[:], in_=rb[64:65, :])

                def s2():
                    nc.vector.reciprocal_approx_fast(rbr[:], rbs[:])
                    nc.sync.dma_start(out=rrow, in_=rbr[:])
                    nc.sync.dma_start(out=rb[0:64, :],
                                      in_=rrow.to_broadcast([64, QBS]))

                def s3():
                    if lo == 0:
                        nc.vector.tensor_mul(o2_sb[(qb, j)][0:64, :],
                                             o_ps[0:DK, :], rb[0:64, :])
                    else:
                        osm = rb_pool.tile([64, QBS], bf16, tag="osm", bufs=1, name="osm")
                        nc.vector.tensor_mul(osm[:], o_ps[0:DK, :], rb[0:64, :])
                        nc.sync.dma_start(out=o2_sb[(qb, j)][64:128, :], in_=osm[:])

                return [s1, s2, s3]

            attnv_q = []
            pending_epi = []

            def emit_attnv():
                attnv_q.pop(0)()

            for qb in range(QB):
                for h in range(H):
                    j, lo = h // 2, (h % 2) * 64
                    o_ps = psum_pool.tile([DK + 1, QBS], mybir.dt.float32,
                                          tag=f"o{h % 2}", bufs=1, name="o")
                    inj_units = list(inj[qb][h])
                    inj_at = {8 + 2 * i: u for i, u in enumerate(inj_units)}
                    epi = list(pending_epi)
                    pending_epi = []
                    epi_at = {pos: s for pos, s in zip((3, 4, 6), epi)}

                    for c in range(NCH):
                        if c % 2 == 0:
                            praw = praw_pool.tile([128, 2 * QBS], bf16,
                                                  tag="praw", name="praw")
                        s_ps = psum_pool.tile([128, QBS], mybir.dt.float32,
                                              tag="s", name="s")
                        for s in range(2):
                            nc.tensor.matmul(
                                s_ps[:, s * 512:(s + 1) * 512],
                                kpad[j][lo:lo + 64, c * 128:(c + 1) * 128],
                                qT_sb[j][lo:lo + 64,
                                         qb * QBS + s * 512: qb * QBS + (s + 1) * 512],
                                start=True, stop=True,
                            )
                        hsl = slice((c % 2) * QBS, (c % 2) * QBS + QBS)
                        nc.scalar.activation(praw[:, hsl], s_ps[:], Exp,
                                             bias=0.0, scale=0.125)
                        if c % 2 == 1:
                            # one paired [128, 2048] mask multiply per 2 chunks
                            p_m = pm_pool.tile([128, 2 * QBS], bf16, tag="pm", name="pm")
                            nc.vector.tensor_mul(p_m[:], praw[:],
                                                 mask_t[(qb, c // 2)][:])

                            def mk_attnv(o_ps=o_ps, h=h, c0=c - 1, p_m=p_m):
                                def go(cc):
                                    for s in range(2):
                                        nc.tensor.matmul(
                                            o_ps[:, s * 512:(s + 1) * 512],
                                            vaug[cc][:, h * (DK + 1):(h + 1) * (DK + 1)],
                                            p_m[:, (cc - c0) * QBS + s * 512:
                                                   (cc - c0) * QBS + (s + 1) * 512],
                                            start=(cc == 0), stop=(cc == NCH - 1),
                                        )
                                return [lambda cc=cc: go(cc) for cc in (c0, c0 + 1)]
                            attnv_q.extend(mk_attnv())

                        # drain before epilogue stages: s1/s3 read the previous
                        # head's o_ps, which needs its attnV(15) emitted first
                        while len(attnv_q) > LAG + (1 - c % 2):
                            emit_attnv()
                        if c in epi_at:
                            epi_at[c]()
                        if c in inj_at:
                            u = inj_at[c]
                            if u[0] == "p3":
                                p3_unit(u[1], u[2], ptag=f"o{(h + 1) % 2}")
                            else:
                                qk_unit(u[0], u[1], u[2], ptag=f"o{(h + 1) % 2}")
                        if qb == 0 and h == H - 1 and c == 10:
                            load_masks(1, range(0, 1))

                    pending_epi = epilogue_stages(qb, h, o_ps)

                if qb == 0:
                    load_masks(1, range(1, NCH // 2))

            # ---------------- tail: drain + P3 for qb=1 ----------------
            while attnv_q:
                emit_attnv()
            st1, st2, st3 = pending_epi
            # hoist p3(1,0)'s j<3 matmuls ahead of the last epilogue; keep
            # the PE warm across the epilogue's DMA-latency chain
            y0 = p3_unit(1, 0, ptag="o0", js=range(3), finish=False)
            st1()
            warm(5)
            st2()
            warm(5)
            st3()
            p3_unit(1, 0, ptag="o0", js=[3], y_ps=y0)
            for i in range(1, NFT):
                p3_unit(1, i, ptag=f"o{i % 2}")

    nc.compile()
    return nc


def _get_nc():
    if "nc" not in _cache:
        _cache["nc"] = _build_nc()
    return _cache["nc"]


def _make_in_maps(inputs):
    query = np.asarray(inputs["query"], np.float32)
    key = np.asarray(inputs["key"], np.float32)
    value = np.asarray(inputs["value"], np.float32)
    mask = np.asarray(inputs["mask"], bool)
    shared = {
        "wqT": np.ascontiguousarray(np.asarray(inputs["Wq"], np.float32).T).astype(BF16),
        "wkT": np.ascontiguousarray(np.asarray(inputs["Wk"], np.float32).T).astype(BF16),
        "wvT": np.ascontiguousarray(np.asarray(inputs["Wv"], np.float32).T).astype(BF16),
        "woT": np.ascontiguousarray(np.asarray(inputs["Wo"], np.float32).T).astype(BF16),
        "bq": np.asarray(inputs["bq"], np.float32),
        "bk": np.asarray(inputs["bk"], np.float32),
        "bv": np.asarray(inputs["bv"], np.float32),
        "bo": np.asarray(inputs["bo"], np.float32),
    }
    in_maps = []
    for b in range(N_CORES):
        m = dict(shared)
        m["xqT"] = np.ascontiguousarray(query[b].T).astype(BF16)
        m["xkT"] = np.ascontiguousarray(key[b].T).astype(BF16)
        m["xvT"] = np.ascontiguousarray(value[b].T).astype(BF16)
        mb = (~mask[b]).T.astype(BF16)          # (1 - mask)^T, [t2, q]
        m["mbar"] = np.ascontiguousarray(mb.reshape(NCH, 128, T))
        in_maps.append(m)
    return in_maps


def run(inputs, trace=False, **kwargs):
    from concourse.bass_utils import run_bass_kernel_spmd
    nc = _get_nc()
    res = run_bass_kernel_spmd(nc, _make_in_maps(inputs),
                               core_ids=list(range(N_CORES)),
                               trace=trace, **kwargs)
    y = np.stack([np.asarray(res.results[b]["yT"]).astype(np.float32).T
                  for b in range(N_CORES)])
    return y, res


def kernel(**inputs) -> np.ndarray:
    y, _ = run(inputs, trace=False)
    return y


# revision 14
# speedup vs baseline: 1.2668x; 1.2668x over previous
"""Multi-head attention (B=8, T=2048, D=512, H=8) on 8 TRN2 NeuronCores.

Sharding: data-parallel over batch - one batch element per core, no
collectives. Host-side prep: transpose x inputs to [D, T], cast matmul
operands to bf16, pass (1 - mask)^T chunk-major, transpose per-core y^T
back to [T, D] (y travels bf16; host converts to f32).

The kernel is organized around keeping the Scalar (activation) engine's
exp stream gapless: 256 exp slices of [128, 1024] at ~1us each are the
hard floor. Everything else (projections, attn-V, output proj,
epilogues) is scheduled into the PE/DVE/DMA slack around that stream:

  P1: PE p-state warmup, then V = x Wv^T + bv -> vaug tiles (ones
      column for the softmax denominator), with K j=0,1 and Q j=0
      projections interleaved as their DMAs land.
  P2: one global pipeline over (qb, h, c):
        S^T[t2,q] = K_h^T.T @ Q_h^T       (64-contraction matmul at
                                           partition base (h%2)*64; PSUM
                                           "s" 2-slot rotation)
        P_raw     = exp(S^T / 8)          (ScalarE, the critical stream,
                                           into [128,2048] chunk pairs)
        P         = P_raw * (1-mask)^T    (DVE, one paired [128,2048]
                                           multiply per 2 chunks)
        O_aug^T  += Vaug_h.T @ P          (PSUM "o" banks, global 3-chunk
                                           lag via a FIFO)
      Remaining K/Q projections and P3(qb=0) y-tiles are injected into
      the "o"-parity PSUM banks mid-head so they never stall the "s"
      rotation. The head epilogue (approx-reciprocal of the denominator
      row + normalize into o2) is emitted as 3 stages interleaved into
      the NEXT head's chunks so DVE never idles on its DMA-latency chain.
  P3: y^T = Wo^T.T @ O^T (+bo) per fo-tile; qb=0's tiles injected during
      qb=1's middle heads; qb=1's in the tail with the j<3 partial
      matmuls hoisted before the final epilogue and keep-warm matmuls
      covering the epilogue's DMA latency.

No max-subtraction in softmax: scores are O(6) so exp is safe in f32.
"""

import numpy as np
import ml_dtypes

B, T, FDIM, H = 8, 2048, 512, 8
DK = FDIM // H          # 64
NFT = FDIM // 128       # 4 fo-tiles
NCH = T // 128          # 16 t2-chunks
QB = 2                  # q blocks
QBS = T // QB           # 1024
N_CORES = 8
LAG = 3                 # attnV trails the exp stream by LAG chunks (global)

BF16 = ml_dtypes.bfloat16

_cache = {}


def _build_nc():
    import concourse.bass as bass
    import concourse.mybir as mybir
    from concourse import bacc, tile

    f32 = mybir.dt.float32
    bf16 = mybir.dt.bfloat16
    Exp = mybir.ActivationFunctionType.Exp
    Alu = mybir.AluOpType

    nc = bacc.Bacc("TRN2", target_bir_lowering=False, debug=False,
                   num_devices=N_CORES)

    # DRAM I/O (per-core shard shapes)
    xqT = nc.dram_tensor("xqT", [FDIM, T], bf16, kind="ExternalInput")
    xkT = nc.dram_tensor("xkT", [FDIM, T], bf16, kind="ExternalInput")
    xvT = nc.dram_tensor("xvT", [FDIM, T], bf16, kind="ExternalInput")
    wqT = nc.dram_tensor("wqT", [FDIM, FDIM], bf16, kind="ExternalInput")
    wkT = nc.dram_tensor("wkT", [FDIM, FDIM], bf16, kind="ExternalInput")
    wvT = nc.dram_tensor("wvT", [FDIM, FDIM], bf16, kind="ExternalInput")
    woT = nc.dram_tensor("woT", [FDIM, FDIM], bf16, kind="ExternalInput")
    bq = nc.dram_tensor("bq", [FDIM], f32, kind="ExternalInput")
    bk = nc.dram_tensor("bk", [FDIM], f32, kind="ExternalInput")
    bv = nc.dram_tensor("bv", [FDIM], f32, kind="ExternalInput")
    bo = nc.dram_tensor("bo", [FDIM], f32, kind="ExternalInput")
    mbar = nc.dram_tensor("mbar", [NCH, 128, T], bf16, kind="ExternalInput")
    yT = nc.dram_tensor("yT", [FDIM, T], bf16, kind="ExternalOutput")
    # DRAM bounce rows for partition-broadcasting softmax reciprocals
    rscratch = nc.dram_tensor("rscratch", [QB * H, QBS], f32)

    with tile.TileContext(nc) as tc:
        with (
            tc.tile_pool(name="consts", bufs=1) as consts,
            tc.tile_pool(name="qt", bufs=1) as qt_pool,
            tc.tile_pool(name="kpad", bufs=1) as kpad_pool,
            tc.tile_pool(name="vaug", bufs=1) as vaug_pool,
            tc.tile_pool(name="osb", bufs=1) as osb_pool,
            tc.tile_pool(name="ysb", bufs=1) as ysb_pool,
            tc.tile_pool(name="xt", bufs=4) as xt_pool,
            tc.tile_pool(name="mask", bufs=8) as mask_pool,
            tc.tile_pool(name="praw", bufs=2) as praw_pool,
            tc.tile_pool(name="pm", bufs=3) as pm_pool,
            tc.tile_pool(name="rb", bufs=1) as rb_pool,
            tc.tile_pool(name="psum", bufs=2, space="PSUM") as psum_pool,
        ):
            # ---- DMA staging, critical path first ----
            wv_sb = [consts.tile([128, FDIM], bf16, tag=f"wv{fc}", name=f"wv{fc}") for fc in range(4)]
            wk_sb = [consts.tile([128, FDIM], bf16, tag=f"wk{fc}", name=f"wk{fc}") for fc in range(4)]
            wq_sb = [consts.tile([128, FDIM], bf16, tag=f"wq{fc}", name=f"wq{fc}") for fc in range(4)]
            wo_sb = [consts.tile([128, FDIM], bf16, tag=f"wo{j}", name=f"wo{j}") for j in range(NFT)]
            bq_sb = consts.tile([128, NFT], f32, tag="bq", name="bq")
            bk_sb = consts.tile([128, NFT], f32, tag="bk", name="bk")
            bo_sb = consts.tile([128, NFT], f32, tag="bo", name="bo")
            bv_bcast = consts.tile([128, FDIM], f32, tag="bv_bcast", name="bv_bcast")

            for fc in range(4):
                nc.sync.dma_start(out=wv_sb[fc][:], in_=wvT[fc * 128:(fc + 1) * 128, :])
            nc.sync.dma_start(
                out=bv_bcast[:],
                in_=bv.ap().rearrange("(a f) -> a f", a=1).to_broadcast([128, FDIM]))
            xts_v = []
            for fc in range(4):
                xt = xt_pool.tile([128, T], bf16, tag="xv", bufs=4, name="xv")
                nc.sync.dma_start(out=xt[:], in_=xvT[fc * 128:(fc + 1) * 128, :])
                xts_v.append(xt)
            for fc in range(4):
                nc.sync.dma_start(out=wk_sb[fc][:], in_=wkT[fc * 128:(fc + 1) * 128, :])
            xts_k = []
            for fc in range(4):
                xt = xt_pool.tile([128, T], bf16, tag="xk", bufs=4, name="xk")
                nc.sync.dma_start(out=xt[:], in_=xkT[fc * 128:(fc + 1) * 128, :])
                xts_k.append(xt)
            for b_dram, b_t in ((bk, bk_sb), (bq, bq_sb)):
                nc.sync.dma_start(out=b_t[:], in_=b_dram.ap().rearrange("(j p) -> p j", p=128))
            for fc in range(4):
                nc.sync.dma_start(out=wq_sb[fc][:], in_=wqT[fc * 128:(fc + 1) * 128, :])
            xts_q = []
            for fc in range(4):
                xt = xt_pool.tile([128, T], bf16, tag="xq", bufs=4, name="xq")
                nc.sync.dma_start(out=xt[:], in_=xqT[fc * 128:(fc + 1) * 128, :])
                xts_q.append(xt)
            nc.sync.dma_start(out=bo_sb[:], in_=bo.ap().rearrange("(j p) -> p j", p=128))
            for j in range(NFT):
                nc.sync.dma_start(out=wo_sb[j][:], in_=woT[j * 128:(j + 1) * 128, :])

            # ---- persistent activation tiles ----
            # kpad[h]: per-head K^T in a zero-padded 128-row tile (the zero
            # half cancels the sibling head's Q rows). Full-row stationary
            # tiles keep the PE ldweights/matmul pipeline overlapped - a
            # 64-row tile-positioned variant runs ~1.7x slower per matmul.
            qT_sb = [qt_pool.tile([128, T], bf16, tag=f"qT{j}", name=f"qT{j}") for j in range(NFT)]
            kpad = [kpad_pool.tile([128, T], bf16, tag=f"kp{h}", name=f"kp{h}") for h in range(H)]
            vaug = [vaug_pool.tile([128, H * (DK + 1)], bf16, tag=f"va{tt}", name=f"va{tt}")
                    for tt in range(NCH)]
            for tt in range(NCH):
                va = vaug[tt][:].rearrange("p (h d) -> p h d", d=DK + 1)
                nc.vector.memset(va[:, :, DK:DK + 1], 1.0)
            for h in range(H):
                half = slice(64, 128) if h % 2 == 0 else slice(0, 64)
                nc.vector.memset(kpad[h][half, :], 0.0)

            o2_sb = {}
            for qb in range(QB):
                for j in range(NFT):
                    o2_sb[(qb, j)] = osb_pool.tile([128, QBS], bf16, tag=f"o2_{qb}_{j}",
                                                   name=f"o2_{qb}_{j}")

            # ---------------- unit emitters ----------------
            # PSUM: tag "s" = 2x[128,1024] (4 banks, the exp-feed rotation)
            #       tag o0/o1 = 1 each (2 banks each: o-accum / injected units)

            def warm(n, tag="s"):
                # p-state keep-warm: junk matmuls, results never read
                for _ in range(n):
                    wps = psum_pool.tile([128, 512], mybir.dt.float32,
                                         tag=tag, name="warm")
                    nc.tensor.matmul(wps[:], wv_sb[0][:, 0:128], wv_sb[0][:],
                                     start=True, stop=True)

            def v_unit(tt):
                ps = psum_pool.tile([128, 512], mybir.dt.float32,
                                    tag=f"o{tt % 2}", bufs=1, name="vp")
                for fc in range(4):
                    nc.tensor.matmul(
                        ps[:],
                        xts_v[fc][:, tt * 128:(tt + 1) * 128],
                        wv_sb[fc][:],
                        start=(fc == 0), stop=(fc == 3),
                    )
                va = vaug[tt][:].rearrange("p (h d) -> p h d", d=DK + 1)
                nc.vector.scalar_tensor_tensor(
                    out=va[:, :, 0:DK],
                    in0=ps[:].rearrange("p (h d) -> p h d", d=DK),
                    scalar=1.0,
                    in1=bv_bcast[:].rearrange("p (h d) -> p h d", d=DK),
                    op0=Alu.mult, op1=Alu.add,
                )

            def qk_unit(kind, j, s, ptag):
                xts, w_sb, b_t = ((xts_q, wq_sb, bq_sb) if kind == "q"
                                  else (xts_k, wk_sb, bk_sb))
                ps = psum_pool.tile([128, 512], mybir.dt.float32,
                                    tag=ptag, bufs=(2 if ptag == "s" else 1),
                                    name="qkp")
                for fc in range(4):
                    nc.tensor.matmul(
                        ps[:],
                        w_sb[fc][:, j * 128:(j + 1) * 128],
                        xts[fc][:, s * 512:(s + 1) * 512],
                        start=(fc == 0), stop=(fc == 3),
                    )
                sl = slice(s * 512, (s + 1) * 512)
                if kind == "q":
                    nc.vector.tensor_scalar_add(qT_sb[j][:, sl], ps[:],
                                                b_t[:, j:j + 1])
                else:
                    nc.vector.tensor_scalar_add(
                        kpad[2 * j][0:64, sl], ps[0:64, :], b_t[0:64, j:j + 1])
                    nc.vector.tensor_scalar_add(
                        kpad[2 * j + 1][64:128, sl], ps[64:128, :],
                        b_t[64:128, j:j + 1])

            def p3_unit(qb, i, ptag, js=range(NFT), y_ps=None, finish=True):
                # output-projection fo-tile i for q-block qb over head-pair
                # tiles `js`; returns the psum tile when not finished.
                qsl = slice(qb * QBS, (qb + 1) * QBS)
                if y_ps is None:
                    y_ps = psum_pool.tile([128, QBS], mybir.dt.float32,
                                          tag=ptag, bufs=1, name="y")
                js = list(js)
                for j in js:
                    for s in range(2):
                        nc.tensor.matmul(
                            y_ps[:, s * 512:(s + 1) * 512],
                            wo_sb[j][:, i * 128:(i + 1) * 128],
                            o2_sb[(qb, j)][:, s * 512:(s + 1) * 512],
                            start=(j == 0), stop=(finish and j == NFT - 1),
                        )
                if not finish:
                    return y_ps
                y_sb = ysb_pool.tile([128, QBS], bf16, tag="ysb", name="ysb")
                nc.vector.tensor_scalar_add(y_sb[:], y_ps[:], bo_sb[:, i:i + 1])
                nc.sync.dma_start(out=yT[i * 128:(i + 1) * 128, qsl], in_=y_sb[:])
                return None

            # ---------------- P1 ----------------
            warm(20)
            p1_stream = [("v", tt) for tt in range(NCH)]
            p1_stream[5:5] = [("k", 0, 0), ("k", 0, 1), ("k", 0, 2), ("k", 0, 3)]
            p1_stream[13:13] = [("k", 1, 0), ("k", 1, 1), ("q", 0, 0)]
            p1_stream.extend([("k", 1, 2), ("k", 1, 3), ("q", 0, 1)])
            for u in p1_stream:
                if u[0] == "v":
                    v_unit(u[1])
                else:
                    qk_unit(u[0], u[1], u[2], ptag="s")

            # ---------------- P2 ----------------
            # deadlines: k(j)/q(j, qb-halves) complete before head 2j of
            # that q-block (k(0), k(1), q(0) qb0-half done in P1).
            inj = {qb: {h: [] for h in range(H)} for qb in range(QB)}
            inj[0][0] = [("q", 1, 0), ("q", 1, 1)]
            inj[0][1] = [("k", 2, 0), ("k", 2, 1)]
            inj[0][2] = [("k", 2, 2), ("k", 2, 3)]
            inj[0][3] = [("q", 2, 0), ("q", 2, 1)]
            inj[0][4] = [("k", 3, 0), ("k", 3, 1), ("k", 3, 2)]
            inj[0][5] = [("k", 3, 3), ("q", 3, 0), ("q", 3, 1)]
            inj[0][6] = [("q", 0, 2)]
            inj[0][7] = [("q", 0, 3)]
            inj[1][0] = [("q", 1, 2), ("q", 1, 3)]
            inj[1][1] = [("q", 2, 2), ("q", 2, 3)]
            inj[1][2] = [("q", 3, 2), ("q", 3, 3)]
            inj[1][3] = [("p3", 0, 0)]
            inj[1][4] = [("p3", 0, 1)]
            inj[1][5] = [("p3", 0, 2)]
            inj[1][6] = [("p3", 0, 3)]

            mask_t = {}

            def load_masks(qb, pairs):
                for p in pairs:
                    mt = mask_pool.tile([128, 2 * QBS], bf16, tag="mask", name="mask")
                    for k in range(2):
                        nc.sync.dma_start(
                            out=mt[:, k * QBS:(k + 1) * QBS],
                            in_=mbar[2 * p + k, :, qb * QBS:(qb + 1) * QBS])
                    mask_t[(qb, p)] = mt

            load_masks(0, range(NCH // 2))

            def epilogue_stages(qb, h, o_ps):
                j, lo = h // 2, (h % 2) * 64
                rrow = rscratch.ap()[qb * H + h: qb * H + h + 1, :]
                rb = rb_pool.tile([128, QBS], mybir.dt.float32, tag="rb", bufs=1, name="rb")
                rbs = rb_pool.tile([8, QBS // 8], mybir.dt.float32, tag="rbs", bufs=1, name="rbs")
                rbr = rb_pool.tile([8, QBS // 8], mybir.dt.float32, tag="rbr", bufs=1, name="rbr")

                def s1():
                    nc.vector.tensor_copy(rb[64:65, :], o_ps[DK:DK + 1, :])
                    nc.sync.dma_start(out=rbs[:], in_=rb[64:65, :])

                def s2():
                    nc.vector.reciprocal_approx_fast(rbr[:], rbs[:])
                    nc.sync.dma_start(out=rrow, in_=rbr[:])
                    nc.sync.dma_start(out=rb[0:64, :],
                                      in_=rrow.to_broadcast([64, QBS]))

                def s3():
                    if lo == 0:
                        nc.vector.tensor_mul(o2_sb[(qb, j)][0:64, :],
                                             o_ps[0:DK, :], rb[0:64, :])
                    else:
                        osm = rb_pool.tile([64, QBS], bf16, tag="osm", bufs=1, name="osm")
                        nc.vector.tensor_mul(osm[:], o_ps[0:DK, :], rb[0:64, :])
                        nc.sync.dma_start(out=o2_sb[(qb, j)][64:128, :], in_=osm[:])

                return [s1, s2, s3]

            attnv_q = []
            pending_epi = []

            def emit_attnv():
                attnv_q.pop(0)()

            for qb in range(QB):
                for h in range(H):
                    j, lo = h // 2, (h % 2) * 64
                    o_ps = psum_pool.tile([DK + 1, QBS], mybir.dt.float32,
                                          tag=f"o{h % 2}", bufs=1, name="o")
                    inj_units = list(inj[qb][h])
                    inj_at = {8 + 2 * i: u for i, u in enumerate(inj_units)}
                    epi = list(pending_epi)
                    pending_epi = []
                    epi_at = {pos: s for pos, s in zip((3, 4, 6), epi)}

                    for c in range(NCH):
                        if c % 2 == 0:
                            praw = praw_pool.tile([128, 2 * QBS], bf16,
                                                  tag="praw", name="praw")
                        s_ps = psum_pool.tile([128, QBS], mybir.dt.float32,
                                              tag="s", name="s")
                        for s in range(2):
                            nc.tensor.matmul(
                                s_ps[:, s * 512:(s + 1) * 512],
                                kpad[h][:, c * 128:(c + 1) * 128],
                                qT_sb[j][:, qb * QBS + s * 512: qb * QBS + (s + 1) * 512],
                                start=True, stop=True,
                            )
                        hsl = slice((c % 2) * QBS, (c % 2) * QBS + QBS)
                        nc.scalar.activation(praw[:, hsl], s_ps[:], Exp,
                                             bias=0.0, scale=0.125)
                        if c % 2 == 1:
                            # one paired [128, 2048] mask multiply per 2 chunks
                            p_m = pm_pool.tile([128, 2 * QBS], bf16, tag="pm", name="pm")
                            nc.vector.tensor_mul(p_m[:], praw[:],
                                                 mask_t[(qb, c // 2)][:])

                            def mk_attnv(o_ps=o_ps, h=h, c0=c - 1, p_m=p_m):
                                def go(cc):
                                    for s in range(2):
                                        nc.tensor.matmul(
                                            o_ps[:, s * 512:(s + 1) * 512],
                                            vaug[cc][:, h * (DK + 1):(h + 1) * (DK + 1)],
                                            p_m[:, (cc - c0) * QBS + s * 512:
                                                   (cc - c0) * QBS + (s + 1) * 512],
                                            start=(cc == 0), stop=(cc == NCH - 1),
                                        )
                                return [lambda cc=cc: go(cc) for cc in (c0, c0 + 1)]
                            attnv_q.extend(mk_attnv())

                        # drain before epilogue stages: s1/s3 read the previous
                        # head's o_ps, which needs its attnV(15) emitted first
                        while len(attnv_q) > LAG + (1 - c % 2):
                            emit_attnv()
                        if c in epi_at:
                            epi_at[c]()
                        if c in inj_at:
                            u = inj_at[c]
                            if u[0] == "p3":
                                p3_unit(u[1], u[2], ptag=f"o{(h + 1) % 2}")
                            else:
                                qk_unit(u[0], u[1], u[2], ptag=f"o{(h + 1) % 2}")
                        if qb == 0 and h == H - 1 and c == 10:
                            load_masks(1, range(0, 1))

                    pending_epi = epilogue_stages(qb, h, o_ps)

                if qb == 0:
                    load_masks(1, range(1, NCH // 2))

            # ---------------- tail: drain + P3 for qb=1 ----------------
            while attnv_q:
                emit_attnv()
            st1, st2, st3 = pending_epi
            # hoist p3(1,0)'s j<3 matmuls ahead of the last epilogue; keep
            # the PE warm across the epilogue's DMA-latency chain
            y0 = p3_unit(1, 0, ptag="o0", js=range(3), finish=False)
            st1()
            warm(5)
            st2()
            warm(5)
            st3()
            p3_unit(1, 0, ptag="o0", js=[3], y_ps=y0)
            for i in range(1, NFT):
                p3_unit(1, i, ptag=f"o{i % 2}")

    nc.compile()
    return nc


def _get_nc():
    if "nc" not in _cache:
        _cache["nc"] = _build_nc()
    return _cache["nc"]


def _make_in_maps(inputs):
    query = np.asarray(inputs["query"], np.float32)
    key = np.asarray(inputs["key"], np.float32)
    value = np.asarray(inputs["value"], np.float32)
    mask = np.asarray(inputs["mask"], bool)
    shared = {
        "wqT": np.ascontiguousarray(np.asarray(inputs["Wq"], np.float32).T).astype(BF16),
        "wkT": np.ascontiguousarray(np.asarray(inputs["Wk"], np.float32).T).astype(BF16),
        "wvT": np.ascontiguousarray(np.asarray(inputs["Wv"], np.float32).T).astype(BF16),
        "woT": np.ascontiguousarray(np.asarray(inputs["Wo"], np.float32).T).astype(BF16),
        "bq": np.asarray(inputs["bq"], np.float32),
        "bk": np.asarray(inputs["bk"], np.float32),
        "bv": np.asarray(inputs["bv"], np.float32),
        "bo": np.asarray(inputs["bo"], np.float32),
    }
    in_maps = []
    for b in range(N_CORES):
        m = dict(shared)
        m["xqT"] = np.ascontiguousarray(query[b].T).astype(BF16)
        m["xkT"] = np.ascontiguousarray(key[b].T).astype(BF16)
        m["xvT"] = np.ascontiguousarray(value[b].T).astype(BF16)
        mb = (~mask[b]).T.astype(BF16)          # (1 - mask)^T, [t2, q]
        m["mbar"] = np.ascontiguousarray(mb.reshape(NCH, 128, T))
        in_maps.append(m)
    return in_maps


def run(inputs, trace=False, **kwargs):
    from concourse.bass_utils import run_bass_kernel_spmd
    nc = _get_nc()
    res = run_bass_kernel_spmd(nc, _make_in_maps(inputs),
                               core_ids=list(range(N_CORES)),
                               trace=trace, **kwargs)
    y = np.stack([np.asarray(res.results[b]["yT"]).astype(np.float32).T
                  for b in range(N_CORES)])
    return y, res


def kernel(**inputs) -> np.ndarray:
    y, _ = run(inputs, trace=False)
    return y


# revision 15
# speedup vs baseline: 1.3019x; 1.0277x over previous
"""Multi-head attention (B=8, T=2048, D=512, H=8) on 8 TRN2 NeuronCores.

Sharding: data-parallel over batch - one batch element per core, no
collectives. Host-side prep: transpose x inputs to [D, T], cast matmul
operands to bf16, pass (1 - mask)^T chunk-major, transpose per-core y^T
back to [T, D] (y travels bf16; host converts to f32).

The kernel is organized around keeping the Scalar (activation) engine's
exp stream gapless: 256 exp slices of [128, 1024] at ~1us each are the
hard floor. Everything else (projections, attn-V, output proj,
epilogues) is scheduled into the PE/DVE/DMA slack around that stream:

  P1: PE p-state warmup, then V = x Wv^T + bv -> vaug tiles (ones
      column for the softmax denominator), with K j=0,1 and Q j=0
      projections interleaved as their DMAs land.
  P2: one global pipeline over (qb, h, c):
        S^T[t2,q] = K_h^T.T @ Q_h^T       (64-contraction matmul at
                                           partition base (h%2)*64; PSUM
                                           "s" 2-slot rotation)
        P_raw     = exp(S^T / 8)          (ScalarE, the critical stream,
                                           into [128,2048] chunk pairs)
        P         = P_raw * (1-mask)^T    (DVE, one paired [128,2048]
                                           multiply per 2 chunks)
        O_aug^T  += Vaug_h.T @ P          (PSUM "o" banks, global 3-chunk
                                           lag via a FIFO)
      Remaining K/Q projections and P3(qb=0) y-tiles are injected into
      the "o"-parity PSUM banks mid-head so they never stall the "s"
      rotation. The head epilogue (approx-reciprocal of the denominator
      row + normalize into o2) is emitted as 3 stages interleaved into
      the NEXT head's chunks so DVE never idles on its DMA-latency chain.
  P3: y^T = Wo^T.T @ O^T (+bo) per fo-tile; qb=0's tiles injected during
      qb=1's middle heads; qb=1's in the tail with the j<3 partial
      matmuls hoisted before the final epilogue and keep-warm matmuls
      covering the epilogue's DMA latency.

No max-subtraction in softmax: scores are O(6) so exp is safe in f32.
"""

import numpy as np
import ml_dtypes

B, T, FDIM, H = 8, 2048, 512, 8
DK = FDIM // H          # 64
NFT = FDIM // 128       # 4 fo-tiles
NCH = T // 128          # 16 t2-chunks
QB = 2                  # q blocks
QBS = T // QB           # 1024
N_CORES = 8
LAG = 3                 # attnV trails the exp stream by LAG chunks (global)

BF16 = ml_dtypes.bfloat16

_cache = {}


def _build_nc():
    import concourse.bass as bass
    import concourse.mybir as mybir
    from concourse import bacc, tile

    f32 = mybir.dt.float32
    bf16 = mybir.dt.bfloat16
    Exp = mybir.ActivationFunctionType.Exp
    Alu = mybir.AluOpType

    nc = bacc.Bacc("TRN2", target_bir_lowering=False, debug=False,
                   num_devices=N_CORES)

    # DRAM I/O (per-core shard shapes)
    xqT = nc.dram_tensor("xqT", [FDIM, T], bf16, kind="ExternalInput")
    xkT = nc.dram_tensor("xkT", [FDIM, T], bf16, kind="ExternalInput")
    xvT = nc.dram_tensor("xvT", [FDIM, T], bf16, kind="ExternalInput")
    wqT = nc.dram_tensor("wqT", [FDIM, FDIM], bf16, kind="ExternalInput")
    wkT = nc.dram_tensor("wkT", [FDIM, FDIM], bf16, kind="ExternalInput")
    wvT = nc.dram_tensor("wvT", [FDIM, FDIM], bf16, kind="ExternalInput")
    woT = nc.dram_tensor("woT", [FDIM, FDIM], bf16, kind="ExternalInput")
    bq = nc.dram_tensor("bq", [FDIM], f32, kind="ExternalInput")
    bk = nc.dram_tensor("bk", [FDIM], f32, kind="ExternalInput")
    bv = nc.dram_tensor("bv", [FDIM], f32, kind="ExternalInput")
    bo = nc.dram_tensor("bo", [FDIM], f32, kind="ExternalInput")
    mbar = nc.dram_tensor("mbar", [NCH, 128, T], bf16, kind="ExternalInput")
    yT = nc.dram_tensor("yT", [FDIM, T], bf16, kind="ExternalOutput")
    # DRAM bounce rows for partition-broadcasting softmax reciprocals
    rscratch = nc.dram_tensor("rscratch", [QB * H, QBS], f32)

    with tile.TileContext(nc) as tc:
        with (
            tc.tile_pool(name="consts", bufs=1) as consts,
            tc.tile_pool(name="qt", bufs=1) as qt_pool,
            tc.tile_pool(name="kpad", bufs=1) as kpad_pool,
            tc.tile_pool(name="vaug", bufs=1) as vaug_pool,
            tc.tile_pool(name="osb", bufs=1) as osb_pool,
            tc.tile_pool(name="ysb", bufs=1) as ysb_pool,
            tc.tile_pool(name="xt", bufs=4) as xt_pool,
            tc.tile_pool(name="mask", bufs=8) as mask_pool,
            tc.tile_pool(name="praw", bufs=2) as praw_pool,
            tc.tile_pool(name="pm", bufs=3) as pm_pool,
            tc.tile_pool(name="rb", bufs=1) as rb_pool,
            tc.tile_pool(name="psum", bufs=2, space="PSUM") as psum_pool,
        ):
            # ---- DMA staging, critical path first ----
            wv_sb = [consts.tile([128, FDIM], bf16, tag=f"wv{fc}", name=f"wv{fc}") for fc in range(4)]
            wk_sb = [consts.tile([128, FDIM], bf16, tag=f"wk{fc}", name=f"wk{fc}") for fc in range(4)]
            wq_sb = [consts.tile([128, FDIM], bf16, tag=f"wq{fc}", name=f"wq{fc}") for fc in range(4)]
            wo_sb = [consts.tile([128, FDIM], bf16, tag=f"wo{j}", name=f"wo{j}") for j in range(NFT)]
            bq_sb = consts.tile([128, NFT], f32, tag="bq", name="bq")
            bk_sb = consts.tile([128, NFT], f32, tag="bk", name="bk")
            bo_sb = consts.tile([128, NFT], f32, tag="bo", name="bo")
            bv_bcast = consts.tile([128, FDIM], f32, tag="bv_bcast", name="bv_bcast")

            for fc in range(4):
                nc.sync.dma_start(out=wv_sb[fc][:], in_=wvT[fc * 128:(fc + 1) * 128, :])
            nc.sync.dma_start(
                out=bv_bcast[:],
                in_=bv.ap().rearrange("(a f) -> a f", a=1).to_broadcast([128, FDIM]))
            xts_v = []
            for fc in range(4):
                xt = xt_pool.tile([128, T], bf16, tag="xv", bufs=4, name="xv")
                nc.sync.dma_start(out=xt[:], in_=xvT[fc * 128:(fc + 1) * 128, :])
                xts_v.append(xt)
            for fc in range(4):
                nc.sync.dma_start(out=wk_sb[fc][:], in_=wkT[fc * 128:(fc + 1) * 128, :])
            xts_k = []
            for fc in range(4):
                xt = xt_pool.tile([128, T], bf16, tag="xk", bufs=4, name="xk")
                nc.sync.dma_start(out=xt[:], in_=xkT[fc * 128:(fc + 1) * 128, :])
                xts_k.append(xt)
            for b_dram, b_t in ((bk, bk_sb), (bq, bq_sb)):
                nc.sync.dma_start(out=b_t[:], in_=b_dram.ap().rearrange("(j p) -> p j", p=128))
            for fc in range(4):
                nc.sync.dma_start(out=wq_sb[fc][:], in_=wqT[fc * 128:(fc + 1) * 128, :])
            xts_q = []
            for fc in range(4):
                xt = xt_pool.tile([128, T], bf16, tag="xq", bufs=4, name="xq")
                nc.sync.dma_start(out=xt[:], in_=xqT[fc * 128:(fc + 1) * 128, :])
                xts_q.append(xt)
            nc.sync.dma_start(out=bo_sb[:], in_=bo.ap().rearrange("(j p) -> p j", p=128))
            for j in range(NFT):
                nc.sync.dma_start(out=wo_sb[j][:], in_=woT[j * 128:(j + 1) * 128, :])

            # ---- persistent activation tiles ----
            # kpad[h]: per-head K^T in a zero-padded 128-row tile (the zero
            # half cancels the sibling head's Q rows). Full-row stationary
            # tiles keep the PE ldweights/matmul pipeline overlapped - a
            # 64-row tile-positioned variant runs ~1.7x slower per matmul.
            qT_sb = [qt_pool.tile([128, T], bf16, tag=f"qT{j}", name=f"qT{j}") for j in range(NFT)]
            kpad = [kpad_pool.tile([128, T], bf16, tag=f"kp{h}", name=f"kp{h}") for h in range(H)]
            vaug = [vaug_pool.tile([128, H * (DK + 1)], bf16, tag=f"va{tt}", name=f"va{tt}")
                    for tt in range(NCH)]
            for tt in range(NCH):
                va = vaug[tt][:].rearrange("p (h d) -> p h d", d=DK + 1)
                nc.vector.memset(va[:, :, DK:DK + 1], 1.0)
            for h in range(H):
                half = slice(64, 128) if h % 2 == 0 else slice(0, 64)
                nc.gpsimd.memset(kpad[h][half, :], 0.0)

            o2_sb = {}
            for qb in range(QB):
                for j in range(NFT):
                    o2_sb[(qb, j)] = osb_pool.tile([128, QBS], bf16, tag=f"o2_{qb}_{j}",
                                                   name=f"o2_{qb}_{j}")

            # ---------------- unit emitters ----------------
            # PSUM: tag "s" = 2x[128,1024] (4 banks, the exp-feed rotation)
            #       tag o0/o1 = 1 each (2 banks each: o-accum / injected units)

            def warm(n, tag="s"):
                # p-state keep-warm: junk matmuls, results never read
                for _ in range(n):
                    wps = psum_pool.tile([128, 512], mybir.dt.float32,
                                         tag=tag, name="warm")
                    nc.tensor.matmul(wps[:], wv_sb[0][:, 0:128], wv_sb[0][:],
                                     start=True, stop=True)

            def v_unit(tt, ptag):
                ps = psum_pool.tile([128, 512], mybir.dt.float32,
                                    tag=ptag, bufs=(2 if ptag == "s" else 1),
                                    name="vp")
                for fc in range(4):
                    nc.tensor.matmul(
                        ps[:],
                        xts_v[fc][:, tt * 128:(tt + 1) * 128],
                        wv_sb[fc][:],
                        start=(fc == 0), stop=(fc == 3),
                    )
                va = vaug[tt][:].rearrange("p (h d) -> p h d", d=DK + 1)
                nc.vector.scalar_tensor_tensor(
                    out=va[:, :, 0:DK],
                    in0=ps[:].rearrange("p (h d) -> p h d", d=DK),
                    scalar=1.0,
                    in1=bv_bcast[:].rearrange("p (h d) -> p h d", d=DK),
                    op0=Alu.mult, op1=Alu.add,
                )

            def qk_unit(kind, j, s, ptag):
                xts, w_sb, b_t = ((xts_q, wq_sb, bq_sb) if kind == "q"
                                  else (xts_k, wk_sb, bk_sb))
                ps = psum_pool.tile([128, 512], mybir.dt.float32,
                                    tag=ptag, bufs=(2 if ptag == "s" else 1),
                                    name="qkp")
                for fc in range(4):
                    nc.tensor.matmul(
                        ps[:],
                        w_sb[fc][:, j * 128:(j + 1) * 128],
                        xts[fc][:, s * 512:(s + 1) * 512],
                        start=(fc == 0), stop=(fc == 3),
                    )
                sl = slice(s * 512, (s + 1) * 512)
                if kind == "q":
                    nc.vector.tensor_scalar_add(qT_sb[j][:, sl], ps[:],
                                                b_t[:, j:j + 1])
                else:
                    nc.vector.tensor_scalar_add(
                        kpad[2 * j][0:64, sl], ps[0:64, :], b_t[0:64, j:j + 1])
                    nc.vector.tensor_scalar_add(
                        kpad[2 * j + 1][64:128, sl], ps[64:128, :],
                        b_t[64:128, j:j + 1])

            def p3_unit(qb, i, ptag, js=range(NFT), y_ps=None, finish=True):
                # output-projection fo-tile i for q-block qb over head-pair
                # tiles `js`; returns the psum tile when not finished.
                qsl = slice(qb * QBS, (qb + 1) * QBS)
                if y_ps is None:
                    y_ps = psum_pool.tile([128, QBS], mybir.dt.float32,
                                          tag=ptag, bufs=1, name="y")
                js = list(js)
                for j in js:
                    for s in range(2):
                        nc.tensor.matmul(
                            y_ps[:, s * 512:(s + 1) * 512],
                            wo_sb[j][:, i * 128:(i + 1) * 128],
                            o2_sb[(qb, j)][:, s * 512:(s + 1) * 512],
                            start=(j == 0), stop=(finish and j == NFT - 1),
                        )
                if not finish:
                    return y_ps
                y_sb = ysb_pool.tile([128, QBS], bf16, tag="ysb", name="ysb")
                nc.vector.tensor_scalar_add(y_sb[:], y_ps[:], bo_sb[:, i:i + 1])
                nc.sync.dma_start(out=yT[i * 128:(i + 1) * 128, qsl], in_=y_sb[:])
                return None

            # ---------------- P1 ----------------
            warm(20)
            p1_stream = [("v", tt) for tt in range(NCH)]
            p1_stream[5:5] = [("k", 0, 0), ("k", 0, 1)]
            p1_stream[8:8] = [("k", 0, 2), ("k", 0, 3)]
            p1_stream[11:11] = [("k", 1, 0), ("k", 1, 1)]
            p1_stream[15:15] = [("k", 1, 2), ("k", 1, 3)]
            p1_stream[19:19] = [("k", 2, 0), ("k", 2, 1)]
            p1_stream[23:23] = [("k", 2, 2), ("k", 2, 3)]
            p1_stream.extend([("q", 0, 0), ("q", 0, 1)])
            # rotate P1 units over 4 psum slots (s x2, o0, o1) so the DVE
            # eviction of unit n never stalls unit n+2's matmuls
            rot = ["o0", "s", "o1", "s"]
            for i, u in enumerate(p1_stream):
                if u[0] == "v":
                    v_unit(u[1], rot[i % 4])
                else:
                    qk_unit(u[0], u[1], u[2], ptag=rot[i % 4])

            # ---------------- P2 ----------------
            # deadlines: k(j)/q(j, qb-halves) complete before head 2j of
            # that q-block (k(0), k(1), q(0) qb0-half done in P1).
            inj = {qb: {h: [] for h in range(H)} for qb in range(QB)}
            inj[0][0] = [("q", 1, 0), ("q", 1, 1)]
            inj[0][1] = [("q", 2, 0), ("q", 2, 1)]
            inj[0][2] = [("k", 3, 0), ("k", 3, 1)]
            inj[0][3] = [("k", 3, 2), ("k", 3, 3)]
            inj[0][4] = [("q", 3, 0), ("q", 3, 1)]
            inj[0][5] = [("q", 0, 2)]
            inj[0][6] = [("q", 0, 3)]
            inj[1][0] = [("q", 1, 2), ("q", 1, 3)]
            inj[1][1] = [("q", 2, 2), ("q", 2, 3)]
            inj[1][2] = [("q", 3, 2), ("q", 3, 3)]
            inj[1][3] = [("p3", 0, 0)]
            inj[1][4] = [("p3", 0, 1)]
            inj[1][5] = [("p3", 0, 2)]
            inj[1][6] = [("p3", 0, 3)]

            mask_t = {}

            def load_masks(qb, pairs):
                for p in pairs:
                    mt = mask_pool.tile([128, 2 * QBS], bf16, tag="mask", name="mask")
                    for k in range(2):
                        nc.sync.dma_start(
                            out=mt[:, k * QBS:(k + 1) * QBS],
                            in_=mbar[2 * p + k, :, qb * QBS:(qb + 1) * QBS])
                    mask_t[(qb, p)] = mt

            load_masks(0, range(NCH // 2))

            def epilogue_stages(qb, h, o_ps):
                j, lo = h // 2, (h % 2) * 64
                rrow = rscratch.ap()[qb * H + h: qb * H + h + 1, :]
                rb = rb_pool.tile([128, QBS], mybir.dt.float32, tag="rb", bufs=1, name="rb")
                rbs = rb_pool.tile([8, QBS // 8], mybir.dt.float32, tag="rbs", bufs=1, name="rbs")
                rbr = rb_pool.tile([8, QBS // 8], mybir.dt.float32, tag="rbr", bufs=1, name="rbr")

                def s1():
                    nc.vector.tensor_copy(rb[64:65, :], o_ps[DK:DK + 1, :])
                    nc.sync.dma_start(out=rbs[:], in_=rb[64:65, :])

                def s2():
                    nc.vector.reciprocal_approx_fast(rbr[:], rbs[:])
                    nc.sync.dma_start(out=rrow, in_=rbr[:])
                    nc.sync.dma_start(out=rb[0:64, :],
                                      in_=rrow.to_broadcast([64, QBS]))

                def s3():
                    if lo == 0:
                        nc.vector.tensor_mul(o2_sb[(qb, j)][0:64, :],
                                             o_ps[0:DK, :], rb[0:64, :])
                    else:
                        osm = rb_pool.tile([64, QBS], bf16, tag="osm", bufs=1, name="osm")
                        nc.vector.tensor_mul(osm[:], o_ps[0:DK, :], rb[0:64, :])
                        nc.sync.dma_start(out=o2_sb[(qb, j)][64:128, :], in_=osm[:])

                return [s1, s2, s3]

            attnv_q = []
            pending_epi = []

            def emit_attnv():
                attnv_q.pop(0)()

            for qb in range(QB):
                for h in range(H):
                    j, lo = h // 2, (h % 2) * 64
                    o_ps = psum_pool.tile([DK + 1, QBS], mybir.dt.float32,
                                          tag=f"o{h % 2}", bufs=1, name="o")
                    inj_units = list(inj[qb][h])
                    inj_at = {8 + 2 * i: u for i, u in enumerate(inj_units)}
                    epi = list(pending_epi)
                    pending_epi = []
                    epi_at = {pos: s for pos, s in zip((3, 4, 6), epi)}

                    for c in range(NCH):
                        if c % 2 == 0:
                            praw = praw_pool.tile([128, 2 * QBS], bf16,
                                                  tag="praw", name="praw")
                        s_ps = psum_pool.tile([128, QBS], mybir.dt.float32,
                                              tag="s", name="s")
                        for s in range(2):
                            nc.tensor.matmul(
                                s_ps[:, s * 512:(s + 1) * 512],
                                kpad[h][:, c * 128:(c + 1) * 128],
                                qT_sb[j][:, qb * QBS + s * 512: qb * QBS + (s + 1) * 512],
                                start=True, stop=True,
                            )
                        hsl = slice((c % 2) * QBS, (c % 2) * QBS + QBS)
                        nc.scalar.activation(praw[:, hsl], s_ps[:], Exp,
                                             bias=0.0, scale=0.125)
                        if c % 2 == 1:
                            # one paired [128, 2048] mask multiply per 2 chunks
                            p_m = pm_pool.tile([128, 2 * QBS], bf16, tag="pm", name="pm")
                            nc.vector.tensor_mul(p_m[:], praw[:],
                                                 mask_t[(qb, c // 2)][:])

                            def mk_attnv(o_ps=o_ps, h=h, c0=c - 1, p_m=p_m):
                                def go(cc):
                                    for s in range(2):
                                        nc.tensor.matmul(
                                            o_ps[:, s * 512:(s + 1) * 512],
                                            vaug[cc][:, h * (DK + 1):(h + 1) * (DK + 1)],
                                            p_m[:, (cc - c0) * QBS + s * 512:
                                                   (cc - c0) * QBS + (s + 1) * 512],
                                            start=(cc == 0), stop=(cc == NCH - 1),
                                        )
                                return [lambda cc=cc: go(cc) for cc in (c0, c0 + 1)]
                            attnv_q.extend(mk_attnv())

                        # drain before epilogue stages: s1/s3 read the previous
                        # head's o_ps, which needs its attnV(15) emitted first
                        while len(attnv_q) > LAG + (1 - c % 2):
                            emit_attnv()
                        if c in epi_at:
                            epi_at[c]()
                        if c in inj_at:
                            u = inj_at[c]
                            if u[0] == "p3":
                                p3_unit(u[1], u[2], ptag=f"o{(h + 1) % 2}")
                            else:
                                qk_unit(u[0], u[1], u[2], ptag=f"o{(h + 1) % 2}")
                        if qb == 0 and h == H - 1 and c == 10:
                            load_masks(1, range(0, 1))

                    pending_epi = epilogue_stages(qb, h, o_ps)

                if qb == 0:
                    load_masks(1, range(1, NCH // 2))

            # ---------------- tail: drain + P3 for qb=1 ----------------
            while attnv_q:
                emit_attnv()
            st1, st2, st3 = pending_epi
            # hoist p3(1,0)'s j<3 matmuls ahead of the last epilogue; keep
            # the PE warm across the epilogue's DMA-latency chain
            y0 = p3_unit(1, 0, ptag="o0", js=range(3), finish=False)
            st1()
            warm(5)
            st2()
            warm(5)
            st3()
            p3_unit(1, 0, ptag="o0", js=[3], y_ps=y0)
            for i in range(1, NFT):
                p3_unit(1, i, ptag=f"o{i % 2}")

    nc.compile()
    return nc


def _get_nc():
    if "nc" not in _cache:
        _cache["nc"] = _build_nc()
    return _cache["nc"]


def _make_in_maps(inputs):
    query = np.asarray(inputs["query"], np.float32)
    key = np.asarray(inputs["key"], np.float32)
    value = np.asarray(inputs["value"], np.float32)
    mask = np.asarray(inputs["mask"], bool)
    shared = {
        "wqT": np.ascontiguousarray(np.asarray(inputs["Wq"], np.float32).T).astype(BF16),
        "wkT": np.ascontiguousarray(np.asarray(inputs["Wk"], np.float32).T).astype(BF16),
        "wvT": np.ascontiguousarray(np.asarray(inputs["Wv"], np.float32).T).astype(BF16),
        "woT": np.ascontiguousarray(np.asarray(inputs["Wo"], np.float32).T).astype(BF16),
        "bq": np.asarray(inputs["bq"], np.float32),
        "bk": np.asarray(inputs["bk"], np.float32),
        "bv": np.asarray(inputs["bv"], np.float32),
        "bo": np.asarray(inputs["bo"], np.float32),
    }
    in_maps = []
    for b in range(N_CORES):
        m = dict(shared)
        m["xqT"] = np.ascontiguousarray(query[b].T).astype(BF16)
        m["xkT"] = np.ascontiguousarray(key[b].T).astype(BF16)
        m["xvT"] = np.ascontiguousarray(value[b].T).astype(BF16)
        mb = (~mask[b]).T.astype(BF16)          # (1 - mask)^T, [t2, q]
        m["mbar"] = np.ascontiguousarray(mb.reshape(NCH, 128, T))
        in_maps.append(m)
    return in_maps


def run(inputs, trace=False, **kwargs):
    from concourse.bass_utils import run_bass_kernel_spmd
    nc = _get_nc()
    res = run_bass_kernel_spmd(nc, _make_in_maps(inputs),
                               core_ids=list(range(N_CORES)),
                               trace=trace, **kwargs)
    y = np.stack([np.asarray(res.results[b]["yT"]).astype(np.float32).T
                  for b in range(N_CORES)])
    return y, res


def kernel(**inputs) -> np.ndarray:
    y, _ = run(inputs, trace=False)
    return y
